# revision 6
# baseline (speedup 1.0000x reference)
"""DGT block (dynamic graph transformer) Bass kernel for Trainium2.

Sharding: 8 cores = 4 batches x 2 query-halves. Each core handles one
batch's feats/pos and one half (2048) of the queries:
  - kNN scores s(n,j) = f_n . f_j - 0.5||f_j||^2  (rank-equivalent to -dist/2)
    computed as float32r on PE (1 cyc/row vs 4 for fp32); top-16 via
    per-512-chunk max8 + max_index8, with the global column index packed
    into the low 12 mantissa bits of each candidate so the merge
    (max8 / match_replace / max8) carries indices along for free.
  - gather table rows [gk | v | gp] in f16 kept in SBUF (24KB/partition);
    fetched per query tile with gpsimd dma_gather(transpose=True,
    SBUF source) so channels land on partitions.
  - attention MLPs as bf16 matmuls with PSUM accumulation; the per-query
    broadcast terms (gq_n, gp_n) enter via an expander matmul and the
    gathered terms (gk_j, gp_j) via a (-I) matmul.
  - per-channel softmax over the 16 neighbors: pairwise-tree adds on DVE,
    weighted-sum multiply on GpSimd to offload the critical DVE engine.
"""

import numpy as np
import ml_dtypes

B, N, K, DP, DM, EPS = 4, 4096, 16, 64, 128, 1e-5
NQ = N // 2            # queries per core
TQ = 128               # queries per tile
NT = NQ // TQ          # tiles per core (16)
PAIR = TQ * K          # pairs per tile (2048)
CH = 512               # top-k scan chunk size
NCH = N // CH          # 8 chunks
ROW = 3 * DM           # gather-table row elems (f16): [gk(128) | v(128) | gp(128)]

UU_ON_POOL = True      # run the ee*(v+pe) multiply on GpSimd instead of DVE

_CACHE = {}

bf16 = ml_dtypes.bfloat16


def _fold_bn(p):
    g, be, m, v = p.astype(np.float64)
    s = g / np.sqrt(v + EPS)
    return (s).astype(np.float32), (be - m * s).astype(np.float32)


def _build_bass():
    import concourse.bass as bass
    import concourse.mybir as mybir
    import concourse.bacc as bacc
    from concourse.tile import TileContext

    dt = mybir.dt
    AF = mybir.ActivationFunctionType
    ALU = mybir.AluOpType
    AX = mybir.AxisListType

    nc = bacc.Bacc("TRN2", target_bir_lowering=False, debug=False, num_devices=8)

    # ---- I/O ----
    def inp(name, shape, dtype):
        return nc.dram_tensor(name, list(shape), dtype, kind="ExternalInput").ap()

    feats_f32 = inp("feats_f32", (DP, N), dt.float32r)
    feats_bf = inp("feats_bf", (DP, N), dt.bfloat16)
    fb_own = inp("fb_own", (DP, NQ), dt.bfloat16)
    lhsT65_d = inp("lhsT65", (DP + 1, NQ), dt.float32r)
    pos_bf = inp("pos_bf", (3, N), dt.bfloat16)
    pos_own = inp("pos_own", (3, NQ), dt.bfloat16)
    w1t_d = inp("W1fT", (DP, DM), dt.bfloat16)
    wkvt_d = inp("WgkvT", (DM, 2 * DM), dt.bfloat16)
    wqt_d = inp("Wg1qT", (DM, DM), dt.bfloat16)
    wd1t_d = inp("Wd1fT", (3, DM), dt.bfloat16)
    wd2t_d = inp("Wd2fT", (DM, DM), dt.bfloat16)
    wg1t_d = inp("Wg1fT", (DM, DM), dt.bfloat16)
    wg2t_d = inp("Wg2fT", (DM, DM), dt.bfloat16)
    w2t_d = inp("W2fT", (DM, DP), dt.bfloat16)
    e_d = inp("E", (TQ, PAIR), dt.bfloat16)
    negi_d = inp("negI", (DM, DM), dt.float16)
    ident_d = inp("ident", (DM, DM), dt.float32)
    choff_d = inp("choff", (TQ, NCH * 8), dt.float32)
    pkmask_d = inp("pkmask", (TQ, NCH * 8), dt.uint32)
    b1_d = inp("b1", (DM, 1), dt.float32)
    bd1_d = inp("bd1", (DM, 1), dt.float32)
    bd2_d = inp("bd2", (DM, 1), dt.float32)
    bg1_d = inp("bg1", (DM, 1), dt.float32)
    bg2_d = inp("bg2", (DM, 1), dt.float32)
    b2_d = inp("b2", (DP, 1), dt.float32)

    out_d = nc.dram_tensor("out", [DP, NQ], dt.float32, kind="ExternalOutput").ap()

    f32, f32r = dt.float32, dt.float32r
    f16, bft, i16, u16, u32 = dt.float16, dt.bfloat16, dt.int16, dt.uint16, dt.uint32

    with TileContext(nc) as tc:
        with (
            tc.tile_pool(name="const", bufs=1) as cpool,
            tc.tile_pool(name="persist", bufs=1) as ppool,
        ):
            # persistent constants
            w1t = cpool.tile_from(w1t_d)
            wkvt = cpool.tile_from(wkvt_d)
            wqt = cpool.tile_from(wqt_d)
            wd1t = cpool.tile_from(wd1t_d)
            wd2t = cpool.tile_from(wd2t_d)
            wg1t = cpool.tile_from(wg1t_d)
            wg2t = cpool.tile_from(wg2t_d)
            w2t = cpool.tile_from(w2t_d)
            emat = cpool.tile_from(e_d)
            negi = cpool.tile_from(negi_d)
            ident = cpool.tile_from(ident_d)
            choff = cpool.tile_from(choff_d)
            pkmask = cpool.tile_from(pkmask_d)
            b1 = cpool.tile_from(b1_d)
            bd1 = cpool.tile_from(bd1_d)
            bd2 = cpool.tile_from(bd2_d)
            bg1 = cpool.tile_from(bg1_d)
            bg2 = cpool.tile_from(bg2_d)
            b2 = cpool.tile_from(b2_d)

            # persistent working tensors
            rhs65 = ppool.tile([DP + 1, N], f32r)
            lhsT65 = ppool.tile_from(lhsT65_d)
            gqT = ppool.tile([TQ, NQ], bft)   # (q, m) blocks per tile
            gpT = ppool.tile([TQ, NQ], bft)
            res_all = ppool.tile([DM, NQ], bft)
            # gather table in SBUF: token n -> partition n%128, rank n//128
            table = ppool.tile([128, N // 128, ROW], f16)

            # ---------------- Phase A: setup ----------------
            with (
                tc.tile_pool(name="setupA", bufs=2) as apool,
                tc.tile_pool(name="xpool", bufs=1) as xpool,
                tc.tile_pool(name="ps_a", bufs=2, space="PSUM") as psa,
                tc.tile_pool(name="ps_b", bufs=1, space="PSUM") as psb,
            ):
                nc.sync.dma_start(out=rhs65[0:DP, :], in_=feats_f32)

                fbt = xpool.tile([DP, N], bft)
                nc.sync.dma_start(out=fbt[:], in_=feats_bf)
                post = xpool.tile([3, N], bft)
                nc.sync.dma_start(out=post[:], in_=pos_bf)
                xob = xpool.tile([DM, NQ], bft)
                fob = xpool.tile([DP, NQ], bft)
                nc.sync.dma_start(out=fob[:], in_=fb_own)
                poso = xpool.tile([3, NQ], bft)
                nc.sync.dma_start(out=poso[:], in_=pos_own)

                ones64 = cpool.tile([DP, 1], f32)
                nc.vector.memset(ones64[:], 1.0)

                # row 64 of rhs65 first: the tile-0 score matmuls (and the
                # whole DVE top-k pipeline) only gate on rhs65/lhsT65, so
                # finishing this before x/table production lets Phase B start
                # ~25us earlier.
                ff = apool.tile([DP, 512], f32, tag="ff")
                for s in range(8):
                    nc.vector.tensor_mul(ff[:], rhs65[0:DP, bass.ts(s, 512)].bitcast(f32),
                                         rhs65[0:DP, bass.ts(s, 512)].bitcast(f32))
                    ps = psb.tile([1, 512], f32, tag="pssq")
                    nc.tensor.matmul(ps[:], ones64[:], ff[:], start=True, stop=True)
                    nc.scalar.activation(rhs65[DP:DP + 1, bass.ts(s, 512)], ps[:],
                                         AF.Copy, bias=0.0, scale=-0.5)
                    ff = apool.tile([DP, 512], f32, tag="ff")

                xfull = xpool.tile([DM, N], bft)
                for s in range(8):
                    ps = psa.tile([DM, 512], f32, tag="psx")
                    nc.tensor.matmul(ps[:], w1t[:], fbt[:, bass.ts(s, 512)],
                                     start=True, stop=True)
                    nc.scalar.activation(xfull[:, bass.ts(s, 512)], ps[:],
                                         AF.Prelu, bias=b1[:], scale=1.0, alpha=0.2)
                # x for own queries (recomputed from the own slice input)
                for s in range(4):
                    ps = psa.tile([DM, 512], f32, tag="psx")
                    nc.tensor.matmul(ps[:], w1t[:], fob[:, bass.ts(s, 512)],
                                     start=True, stop=True)
                    nc.scalar.activation(xob[:, bass.ts(s, 512)], ps[:],
                                         AF.Prelu, bias=b1[:], scale=1.0, alpha=0.2)

                # gather table rows [gk | v | gp] in f16, written straight
                # into the SBUF-resident table (token n at partition n%128,
                # rank n//128).
                for c in range(32):
                    pkv = psa.tile([TQ, 2 * DM], f32, tag="pskv")
                    nc.tensor.matmul(pkv[:], xfull[:, bass.ts(c, TQ)], wkvt[:],
                                     start=True, stop=True)
                    pgp = psb.tile([TQ, DM], f32, tag="psgp")
                    nc.tensor.matmul(pgp[:], post[:, bass.ts(c, TQ)], wd1t[:],
                                     start=True, stop=True)
                    eng = nc.vector if (c % 2 == 0) else nc.scalar
                    if eng is nc.vector:
                        nc.vector.tensor_copy(out=table[:, c, 0:2 * DM], in_=pkv[:])
                        nc.vector.tensor_copy(out=table[:, c, 2 * DM:ROW], in_=pgp[:])
                    else:
                        nc.scalar.activation(table[:, c, 0:2 * DM], pkv[:], AF.Copy)
                        nc.scalar.activation(table[:, c, 2 * DM:ROW], pgp[:], AF.Copy)

                # gqT / gpT for own queries
                for c in range(NT):
                    pq = psb.tile([TQ, DM], f32, tag="psgq")
                    nc.tensor.matmul(pq[:], xob[:, bass.ts(c, TQ)], wqt[:],
                                     start=True, stop=True)
                    nc.vector.tensor_copy(out=gqT[:, bass.ts(c, DM)], in_=pq[:])
                    pp = psb.tile([TQ, DM], f32, tag="psgq")
                    nc.tensor.matmul(pp[:], poso[:, bass.ts(c, TQ)], wd1t[:],
                                     start=True, stop=True)
                    nc.vector.tensor_copy(out=gpT[:, bass.ts(c, DM)], in_=pp[:])

            # ---------------- Phase B: per query tile ----------------
            with (
                tc.tile_pool(name="score", bufs=2) as spool,
                tc.tile_pool(name="gath", bufs=2) as gpool,
                tc.tile_pool(name="pair", bufs=2) as prpool,
                tc.tile_pool(name="wwuu", bufs=1) as wpool,
                tc.tile_pool(name="topk", bufs=3) as kpool,
                tc.tile_pool(name="tree", bufs=2) as tpool,
                tc.tile_pool(name="ps_s", bufs=3, space="PSUM") as pss,
                tc.tile_pool(name="ps_pair", bufs=1, space="PSUM") as psp,
                tc.tile_pool(name="ps_t", bufs=1, space="PSUM") as pst,
            ):
                for t in range(NT):
                    # ---- scores (f32r: 1 cyc/row on PE) ----
                    sc = spool.tile([TQ, N], f32, tag="sc")
                    for s in range(8):
                        ps = pss.tile([TQ, 512], f32, tag="pssc")
                        nc.tensor.matmul(ps[:],
                                         lhsT65[:, bass.ts(t, TQ)],
                                         rhs65[:, bass.ts(s, 512)],
                                         start=True, stop=True)
                        nc.scalar.activation(sc[:, bass.ts(s, 512)], ps[:], AF.Copy)

                    # ---- top-16 with packed indices ----
                    # per 512-chunk: top-8 values + their in-chunk indices
                    cand = kpool.tile([TQ, NCH * 8], f32, tag="cand")
                    for c in range(NCH):
                        nc.vector.max(out=cand[:, bass.ts(c, 8)],
                                      in_=sc[:, bass.ts(c, CH)])
                    lidx = kpool.tile([TQ, NCH * 8], u16, tag="lidx")
                    for c in range(NCH):
                        nc.vector.max_index(out=lidx[:, bass.ts(c, 8)],
                                            in_max=cand[:, bass.ts(c, 8)],
                                            in_values=sc[:, bass.ts(c, CH)])
                    # global index = chunk*512 + local; pack into low 12
                    # mantissa bits of the candidate value
                    lidxf = kpool.tile([TQ, NCH * 8], f32, tag="lidxf")
                    nc.vector.tensor_copy(out=lidxf[:], in_=lidx[:])
                    gidxf = kpool.tile([TQ, NCH * 8], f32, tag="gidxf")
                    nc.vector.tensor_add(gidxf[:], lidxf[:], choff[:])
                    gidxu = kpool.tile([TQ, NCH * 8], u32, tag="gidxu")
                    nc.vector.tensor_copy(out=gidxu[:], in_=gidxf[:])
                    packed = kpool.tile([TQ, NCH * 8], f32, tag="packed")
                    nc.vector.tensor_tensor(out=packed[:].bitcast(u32),
                                            in0=cand[:].bitcast(u32),
                                            in1=pkmask[:], op=ALU.bitwise_and)
                    nc.vector.tensor_tensor(out=packed[:].bitcast(u32),
                                            in0=packed[:].bitcast(u32),
                                            in1=gidxu[:], op=ALU.bitwise_or)
                    # merge: top-8, kill them, next top-8
                    v8a = kpool.tile([TQ, 8], f32, tag="v8a")
                    nc.vector.max(out=v8a[:], in_=packed[:])
                    repl = kpool.tile([TQ, NCH * 8], f32, tag="repl")
                    nc.vector.match_replace(out=repl[:], in_to_replace=v8a[:],
                                            in_values=packed[:], imm_value=-1e30)
                    v8b = kpool.tile([TQ, 8], f32, tag="v8b")
                    nc.vector.max(out=v8b[:], in_=repl[:])
                    # extract indices from the low bits, replicate into all 8
                    # core groups, transpose so idx16[16c+j, q] = neighbor j.
                    pk16u = kpool.tile([TQ, 16], u32, tag="pk16u")
                    nc.vector.tensor_scalar(out=pk16u[:, 0:8],
                                            in0=v8a[:].bitcast(u32),
                                            scalar1=0xFFF, scalar2=None,
                                            op0=ALU.bitwise_and)
                    nc.vector.tensor_scalar(out=pk16u[:, 8:16],
                                            in0=v8b[:].bitcast(u32),
                                            scalar1=0xFFF, scalar2=None,
                                            op0=ALU.bitwise_and)
                    idxf = kpool.tile([TQ, DM], f32, tag="idxf")
                    nc.vector.tensor_copy(out=idxf[:, 0:16], in_=pk16u[:])
                    nc.vector.tensor_copy(out=idxf[:, 16:32], in_=idxf[:, 0:16])
                    nc.vector.tensor_copy(out=idxf[:, 32:64], in_=idxf[:, 0:32])
                    nc.vector.tensor_copy(out=idxf[:, 64:128], in_=idxf[:, 0:64])
                    pt = pst.tile([DM, TQ], f32, tag="pst")
                    nc.tensor.transpose(pt[:], idxf[:], ident[:])
                    idx16 = kpool.tile([TQ, TQ], i16, tag="idx16")
                    nc.vector.tensor_copy(out=idx16[:], in_=pt[:])

                    # ---- gather [gk | v | gp] from the SBUF table ----
                    gkv = []
                    for gh in range(4):
                        gt = gpool.tile([DM, 3, 512], f16, tag=f"gkv{gh}")
                        nc.gpsimd.dma_gather(
                            out_ap=gt[:], in_ap=table[:],
                            idxs_ap=idx16[:, bass.ts(gh, 32)],
                            num_idxs=512, num_idxs_reg=512, elem_size=ROW,
                            transpose=True,
                            sbuf_tokens_per_rank=128,
                            sbuf_free_dim_per_rank=ROW * 2)
                        gkv.append(gt)

                    # ---- pe MLP ---- (weight-major order: one LDWEIGHTS per group)
                    zp = psp.tile([DM, PAIR], f32, tag="zpair")
                    for hh in range(4):
                        nc.tensor.matmul(zp[:, bass.ts(hh, 512)], gpT[:, bass.ts(t, TQ)],
                                         emat[:, bass.ts(hh, 512)], start=True, stop=False)
                    for hh in range(4):
                        nc.tensor.matmul(zp[:, bass.ts(hh, 512)], negi[:],
                                         gkv[hh][:, 2, :], start=False, stop=True)
                    h1 = prpool.tile([DM, PAIR], bft, tag="h1")
                    nc.scalar.activation(h1[:], zp[:], AF.Prelu, bias=bd1[:],
                                         scale=1.0, alpha=0.2)
                    zp = psp.tile([DM, PAIR], f32, tag="zpair")
                    for hh in range(4):
                        sl = bass.ts(hh, 512)
                        nc.tensor.matmul(zp[:, sl], wd2t[:], h1[:, sl],
                                         start=True, stop=True)
                    pe = prpool.tile([DM, PAIR], bft, tag="pe")
                    nc.scalar.activation(pe[:], zp[:], AF.Prelu, bias=bd2[:],
                                         scale=1.0, alpha=0.2)

                    # ---- attention MLP ----
                    zp = psp.tile([DM, PAIR], f32, tag="zpair")
                    for hh in range(4):
                        nc.tensor.matmul(zp[:, bass.ts(hh, 512)], gqT[:, bass.ts(t, TQ)],
                                         emat[:, bass.ts(hh, 512)], start=True, stop=False)
                    for hh in range(4):
                        nc.tensor.matmul(zp[:, bass.ts(hh, 512)], negi[:],
                                         gkv[hh][:, 0, :], start=False, stop=False)
                    for hh in range(4):
                        nc.tensor.matmul(zp[:, bass.ts(hh, 512)], wg1t[:],
                                         pe[:, bass.ts(hh, 512)], start=False, stop=True)
                    a1 = prpool.tile([DM, PAIR], bft, tag="a1")
                    nc.scalar.activation(a1[:], zp[:], AF.Prelu, bias=bg1[:],
                                         scale=1.0, alpha=0.2)
                    zp = psp.tile([DM, PAIR], f32, tag="zpair")
                    for hh in range(4):
                        sl = bass.ts(hh, 512)
                        nc.tensor.matmul(zp[:, sl], wg2t[:], a1[:, sl],
                                         start=True, stop=True)
                    a2 = prpool.tile([DM, PAIR], bft, tag="a2")
                    nc.scalar.activation(a2[:], zp[:], AF.Prelu, bias=bg2[:],
                                         scale=1.0, alpha=0.2)
                    ee = prpool.tile([DM, PAIR], bft, tag="ee")
                    nc.scalar.activation(ee[:], a2[:], AF.Exp, bias=0.0,
                                         scale=1.0 / 64.0)

                    # ---- softmax-normalized weighted sum ----
                    # pairwise-tree segment sums (fp32) instead of 1x
                    # TensorReduce
                    def tree_sum(src):
                        e3 = src[:].rearrange("p (q k) -> p q k", k=16)
                        t1 = tpool.tile([DM, TQ * 8], f32, tag="tr1")
                        nc.vector.tensor_tensor(
                            out=t1[:].rearrange("p (q k) -> p q k", k=8),
                            in0=e3[:, :, 0:8], in1=e3[:, :, 8:16], op=ALU.add)
                        t13 = t1[:].rearrange("p (q k) -> p q k", k=8)
                        t2 = tpool.tile([DM, TQ * 4], f32, tag="tr2")
                        nc.vector.tensor_tensor(
                            out=t2[:].rearrange("p (q k) -> p q k", k=4),
                            in0=t13[:, :, 0:4], in1=t13[:, :, 4:8], op=ALU.add)
                        t23 = t2[:].rearrange("p (q k) -> p q k", k=4)
                        t3 = tpool.tile([DM, TQ * 2], f32, tag="tr3")
                        nc.vector.tensor_tensor(
                            out=t3[:].rearrange("p (q k) -> p q k", k=2),
                            in0=t23[:, :, 0:2], in1=t23[:, :, 2:4], op=ALU.add)
                        t33 = t3[:].rearrange("p (q k) -> p q k", k=2)
                        t4 = tpool.tile([DM, TQ], f32, tag="tr4")
                        nc.vector.tensor_tensor(
                            out=t4[:].rearrange("p (q k) -> p q k", k=1),
                            in0=t33[:, :, 0:1], in1=t33[:, :, 1:2], op=ALU.add)
                        return t4

                    ssum = tree_sum(ee)
                    rrec = kpool.tile([DM, TQ], f32, tag="rrec")
                    nc.vector.reciprocal(rrec[:], ssum[:])
                    ww = wpool.tile([DM, PAIR], bft, tag="ww")
                    for hh in range(4):
                        nc.vector.tensor_add(ww[:, bass.ts(hh, 512)],
                                             gkv[hh][:, 1, :],
                                             pe[:, bass.ts(hh, 512)])
                    uu = wpool.tile([DM, PAIR], bft, tag="uu")
                    if UU_ON_POOL:
                        nc.gpsimd.tensor_tensor(out=uu[:], in0=ee[:], in1=ww[:],
                                                op=ALU.mult)
                    else:
                        nc.vector.tensor_mul(uu[:], ee[:], ww[:])
                    ru = tree_sum(uu)
                    nc.vector.tensor_mul(res_all[:, bass.ts(t, TQ)], ru[:], rrec[:])

            # ---------------- Phase C: output ----------------
            with (
                tc.tile_pool(name="outp", bufs=2) as opool,
                tc.tile_pool(name="ps_o", bufs=2, space="PSUM") as pso,
            ):
                o1 = opool.tile([DP, NQ], f32, tag="o1")
                for s in range(4):
                    ps = pso.tile([DP, 512], f32, tag="pso")
                    nc.tensor.matmul(ps[:], w2t[:], res_all[:, bass.ts(s, 512)],
                                     start=True, stop=True)
                    nc.scalar.activation(o1[:, bass.ts(s, 512)], ps[:], AF.Prelu,
                                         bias=b2[:], scale=1.0, alpha=0.2)
                o2 = opool.tile([DP, NQ], f32, tag="o2")
                nc.vector.tensor_add(o2[:], o1[:], lhsT65[0:DP, :].bitcast(f32))
                nc.sync.dma_start(out=out_d, in_=o2[:])

    nc.compile()
    return nc


def _host_prep(inputs):
    """Fold BN into weights, build per-core input maps."""
    s1, b1 = _fold_bn(np.asarray(inputs["bn1"]))
    sd1, bd1 = _fold_bn(np.asarray(inputs["bnd1"]))
    sd2, bd2 = _fold_bn(np.asarray(inputs["bnd2"]))
    sg1, bg1 = _fold_bn(np.asarray(inputs["bng1"]))
    sg2, bg2 = _fold_bn(np.asarray(inputs["bng2"]))
    s2, b2 = _fold_bn(np.asarray(inputs["bn2"]))
    W1f = np.asarray(inputs["W1"]) * s1[:, None]
    Wd1f = np.asarray(inputs["Wd1"]) * sd1[:, None]
    Wd2f = np.asarray(inputs["Wd2"]) * sd2[:, None]
    Wg1f = np.asarray(inputs["Wg1"]) * sg1[:, None]
    Wg2f = np.asarray(inputs["Wg2"]) * sg2[:, None]
    W2f = np.asarray(inputs["W2"]) * s2[:, None]
    Wg1k = (Wg1f @ np.asarray(inputs["Wk"])).astype(np.float32)
    Wg1q = (Wg1f @ np.asarray(inputs["Wq"])).astype(np.float32)
    Wv = np.asarray(inputs["Wv"], np.float32)

    E = np.zeros((TQ, PAIR), np.float32)
    for q in range(TQ):
        E[q, q * K:(q + 1) * K] = 1.0

    choff = np.zeros((TQ, NCH * 8), np.float32)
    for c in range(NCH):
        choff[:, c * 8:(c + 1) * 8] = c * CH

    com = {
        "W1fT": np.ascontiguousarray(W1f.T, dtype=bf16),
        "WgkvT": np.ascontiguousarray(
            np.concatenate([Wg1k.T, Wv.T], axis=1), dtype=bf16),
        "Wg1qT": np.ascontiguousarray(Wg1q.T, dtype=bf16),
        "Wd1fT": np.ascontiguousarray(Wd1f.T, dtype=bf16),
        "Wd2fT": np.ascontiguousarray(Wd2f.T, dtype=bf16),
        "Wg1fT": np.ascontiguousarray(Wg1f.T, dtype=bf16),
        "Wg2fT": np.ascontiguousarray(Wg2f.T, dtype=bf16),
        "W2fT": np.ascontiguousarray(W2f.T, dtype=bf16),
        "E": E.astype(bf16),
        "negI": (-np.eye(DM)).astype(np.float16),
        "ident": np.eye(DM, dtype=np.float32),
        "choff": choff,
        "pkmask": np.full((TQ, NCH * 8), 0xFFFFF000, np.uint32),
        "b1": b1.reshape(DM, 1),
        "bd1": bd1.reshape(DM, 1),
        "bd2": bd2.reshape(DM, 1),
        "bg1": bg1.reshape(DM, 1),
        "bg2": bg2.reshape(DM, 1),
        "b2": b2.reshape(DP, 1),
    }

    feats = np.asarray(inputs["feats"], np.float32)
    pos = np.asarray(inputs["pos"], np.float32)
    in_maps = []
    for c in range(8):
        b, h = c // 2, c % 2
        n0 = h * NQ
        fb = feats[b]
        l65 = np.empty((DP + 1, NQ), np.float32)
        l65[0:DP] = fb[:, n0:n0 + NQ]
        l65[DP] = 1.0
        m = dict(com)
        m["feats_f32"] = np.ascontiguousarray(fb)
        m["feats_bf"] = np.ascontiguousarray(fb, dtype=bf16)
        m["fb_own"] = np.ascontiguousarray(fb[:, n0:n0 + NQ], dtype=bf16)
        m["lhsT65"] = l65
        m["pos_bf"] = np.ascontiguousarray(pos[b], dtype=bf16)
        m["pos_own"] = np.ascontiguousarray(pos[b][:, n0:n0 + NQ], dtype=bf16)
        in_maps.append(m)
    return in_maps


def kernel(**inputs):
    from concourse.bass_utils import run_bass_kernel_spmd

    if "nc" not in _CACHE:
        _CACHE["nc"] = _build_bass()
    nc = _CACHE["nc"]
    in_maps = _host_prep(inputs)
    r = run_bass_kernel_spmd(nc, in_maps, core_ids=list(range(8)),
                             **_CACHE.get("run_kwargs", {}))
    _CACHE["last_result"] = r
    out = np.empty((B, DP, N), np.float32)
    for c in range(8):
        b, h = c // 2, c % 2
        out[b][:, h * NQ:(h + 1) * NQ] = r.results[c]["out"]
    return out


# revision 9
# speedup vs baseline: 1.0971x; 1.0971x over previous
"""DGT block (dynamic graph transformer) Bass kernel for Trainium2.

Sharding: 8 cores = 4 batches x 2 query-halves. Each core handles one
batch's feats/pos and one half (2048) of the queries:
  - kNN scores s(n,j) = f_n . f_j - 0.5||f_j||^2  (rank-equivalent to -dist/2)
    computed as float32r on PE (1 cyc/row vs 4 for fp32); top-16 via
    per-512-chunk max8 + max_index8, with the global column index packed
    into the low 12 mantissa bits of each candidate so the merge
    (max8 / match_replace / max8) carries indices along for free.
  - gather table rows [gk | v | gp] in f16 kept in SBUF (24KB/partition);
    fetched per query tile with gpsimd dma_gather(transpose=True,
    SBUF source) so channels land on partitions.
  - attention MLPs as bf16 matmuls with PSUM accumulation; the per-query
    broadcast terms (gq_n, gp_n) enter via an expander matmul and the
    gathered terms (gk_j, gp_j) via a (-I) matmul.
  - per-channel softmax over the 16 neighbors: pairwise-tree adds on DVE,
    weighted-sum multiply on GpSimd to offload the critical DVE engine.
"""

import numpy as np
import ml_dtypes

B, N, K, DP, DM, EPS = 4, 4096, 16, 64, 128, 1e-5
NQ = N // 2            # queries per core
TQ = 128               # queries per tile
NT = NQ // TQ          # tiles per core (16)
PAIR = TQ * K          # pairs per tile (2048)
CH = 512               # top-k scan chunk size
NCH = N // CH          # 8 chunks
ROW = 3 * DM           # gather-table row elems (f16): [gk(128) | v(128) | gp(128)]

UU_ON_POOL = False     # run the ee*(v+pe) multiply on GpSimd instead of DVE

_CACHE = {}

bf16 = ml_dtypes.bfloat16


def _fold_bn(p):
    g, be, m, v = p.astype(np.float64)
    s = g / np.sqrt(v + EPS)
    return (s).astype(np.float32), (be - m * s).astype(np.float32)


def _build_bass():
    import concourse.bass as bass
    import concourse.mybir as mybir
    import concourse.bacc as bacc
    from concourse.tile import TileContext

    dt = mybir.dt
    AF = mybir.ActivationFunctionType
    ALU = mybir.AluOpType
    AX = mybir.AxisListType

    nc = bacc.Bacc("TRN2", target_bir_lowering=False, debug=False, num_devices=8)

    # ---- I/O ----
    def inp(name, shape, dtype):
        return nc.dram_tensor(name, list(shape), dtype, kind="ExternalInput").ap()

    feats_f32 = inp("feats_f32", (DP, N), dt.float32r)
    feats_bf = inp("feats_bf", (DP, N), dt.bfloat16)
    fb_own = inp("fb_own", (DP, NQ), dt.bfloat16)
    lhsT65_d = inp("lhsT65", (DP + 1, NQ), dt.float32r)
    pos_bf = inp("pos_bf", (3, N), dt.bfloat16)
    pos_own = inp("pos_own", (3, NQ), dt.bfloat16)
    w1t_d = inp("W1fT", (DP, DM), dt.bfloat16)
    wkvt_d = inp("WgkvT", (DM, 2 * DM), dt.bfloat16)
    wqt_d = inp("Wg1qT", (DM, DM), dt.bfloat16)
    wd1t_d = inp("Wd1fT", (3, DM), dt.bfloat16)
    wd2t_d = inp("Wd2fT", (DM, DM), dt.bfloat16)
    wg1t_d = inp("Wg1fT", (DM, DM), dt.bfloat16)
    wg2t_d = inp("Wg2fT", (DM, DM), dt.bfloat16)
    w2t_d = inp("W2fT", (DM, DP), dt.bfloat16)
    e_d = inp("E", (TQ, PAIR // 2), dt.bfloat16)
    negi_d = inp("negI", (DM, DM), dt.float16)
    ident_d = inp("ident", (DM, DM), dt.float32)
    choff_d = inp("choff", (TQ, NCH * 8), dt.float32)
    pkmask_d = inp("pkmask", (TQ, NCH * 8), dt.uint32)
    b1_d = inp("b1", (DM, 1), dt.float32)
    bd1_d = inp("bd1", (DM, 1), dt.float32)
    bd2_d = inp("bd2", (DM, 1), dt.float32)
    bg1_d = inp("bg1", (DM, 1), dt.float32)
    bg2_d = inp("bg2", (DM, 1), dt.float32)
    b2_d = inp("b2", (DP, 1), dt.float32)

    out_d = nc.dram_tensor("out", [DP, NQ], dt.float32, kind="ExternalOutput").ap()

    f32, f32r = dt.float32, dt.float32r
    f16, bft, i16, u16, u32 = dt.float16, dt.bfloat16, dt.int16, dt.uint16, dt.uint32

    with TileContext(nc) as tc:
        with (
            tc.tile_pool(name="const", bufs=1) as cpool,
            tc.tile_pool(name="persist", bufs=1) as ppool,
        ):
            # persistent constants
            w1t = cpool.tile_from(w1t_d)
            wkvt = cpool.tile_from(wkvt_d)
            wqt = cpool.tile_from(wqt_d)
            wd1t = cpool.tile_from(wd1t_d)
            wd2t = cpool.tile_from(wd2t_d)
            wg1t = cpool.tile_from(wg1t_d)
            wg2t = cpool.tile_from(wg2t_d)
            w2t = cpool.tile_from(w2t_d)
            emat = cpool.tile_from(e_d)
            negi = cpool.tile_from(negi_d)
            ident = cpool.tile_from(ident_d)
            choff = cpool.tile_from(choff_d)
            pkmask = cpool.tile_from(pkmask_d)
            b1 = cpool.tile_from(b1_d)
            bd1 = cpool.tile_from(bd1_d)
            bd2 = cpool.tile_from(bd2_d)
            bg1 = cpool.tile_from(bg1_d)
            bg2 = cpool.tile_from(bg2_d)
            b2 = cpool.tile_from(b2_d)

            # persistent working tensors
            rhs65 = ppool.tile([DP + 1, N], f32r)
            lhsT65 = ppool.tile_from(lhsT65_d)
            gqT = ppool.tile([TQ, NQ], bft)   # (q, m) blocks per tile
            gpT = ppool.tile([TQ, NQ], bft)
            res_all = ppool.tile([DM, NQ], bft)
            # gather table in SBUF: token n -> partition n%128, rank n//128
            table = ppool.tile([128, N // 128, ROW], f16)

            # ---------------- Phase A: setup ----------------
            with (
                tc.tile_pool(name="setupA", bufs=2) as apool,
                tc.tile_pool(name="xpool", bufs=1) as xpool,
                tc.tile_pool(name="ps_a", bufs=2, space="PSUM") as psa,
                tc.tile_pool(name="ps_b", bufs=1, space="PSUM") as psb,
            ):
                nc.sync.dma_start(out=rhs65[0:DP, :], in_=feats_f32)

                fbt = xpool.tile([DP, N], bft)
                nc.sync.dma_start(out=fbt[:], in_=feats_bf)
                post = xpool.tile([3, N], bft)
                nc.sync.dma_start(out=post[:], in_=pos_bf)
                xob = xpool.tile([DM, NQ], bft)
                fob = xpool.tile([DP, NQ], bft)
                nc.sync.dma_start(out=fob[:], in_=fb_own)
                poso = xpool.tile([3, NQ], bft)
                nc.sync.dma_start(out=poso[:], in_=pos_own)

                ones64 = cpool.tile([DP, 1], f32)
                nc.vector.memset(ones64[:], 1.0)

                # row 64 of rhs65 first: the tile-0 score matmuls (and the
                # whole DVE top-k pipeline) only gate on rhs65/lhsT65, so
                # finishing this before x/table production lets Phase B start
                # ~25us earlier.
                ff = apool.tile([DP, 512], f32, tag="ff")
                for s in range(8):
                    nc.vector.tensor_mul(ff[:], rhs65[0:DP, bass.ts(s, 512)].bitcast(f32),
                                         rhs65[0:DP, bass.ts(s, 512)].bitcast(f32))
                    ps = psb.tile([1, 512], f32, tag="pssq")
                    nc.tensor.matmul(ps[:], ones64[:], ff[:], start=True, stop=True)
                    nc.scalar.activation(rhs65[DP:DP + 1, bass.ts(s, 512)], ps[:],
                                         AF.Copy, bias=0.0, scale=-0.5)
                    ff = apool.tile([DP, 512], f32, tag="ff")

                xfull = xpool.tile([DM, N], bft)
                for s in range(8):
                    ps = psa.tile([DM, 512], f32, tag="psx")
                    nc.tensor.matmul(ps[:], w1t[:], fbt[:, bass.ts(s, 512)],
                                     start=True, stop=True)
                    nc.scalar.activation(xfull[:, bass.ts(s, 512)], ps[:],
                                         AF.Prelu, bias=b1[:], scale=1.0, alpha=0.2)
                # x for own queries (recomputed from the own slice input)
                for s in range(4):
                    ps = psa.tile([DM, 512], f32, tag="psx")
                    nc.tensor.matmul(ps[:], w1t[:], fob[:, bass.ts(s, 512)],
                                     start=True, stop=True)
                    nc.scalar.activation(xob[:, bass.ts(s, 512)], ps[:],
                                         AF.Prelu, bias=b1[:], scale=1.0, alpha=0.2)

                # gather table rows [gk | v | gp] in f16, written straight
                # into the SBUF-resident table (token n at partition n%128,
                # rank n//128).
                for c in range(32):
                    pkv = psa.tile([TQ, 2 * DM], f32, tag="pskv")
                    nc.tensor.matmul(pkv[:], xfull[:, bass.ts(c, TQ)], wkvt[:],
                                     start=True, stop=True)
                    pgp = psb.tile([TQ, DM], f32, tag="psgp")
                    nc.tensor.matmul(pgp[:], post[:, bass.ts(c, TQ)], wd1t[:],
                                     start=True, stop=True)
                    eng = nc.vector if (c % 2 == 0) else nc.scalar
                    if eng is nc.vector:
                        nc.vector.tensor_copy(out=table[:, c, 0:2 * DM], in_=pkv[:])
                        nc.vector.tensor_copy(out=table[:, c, 2 * DM:ROW], in_=pgp[:])
                    else:
                        nc.scalar.activation(table[:, c, 0:2 * DM], pkv[:], AF.Copy)
                        nc.scalar.activation(table[:, c, 2 * DM:ROW], pgp[:], AF.Copy)

                # gqT / gpT for own queries
                for c in range(NT):
                    pq = psb.tile([TQ, DM], f32, tag="psgq")
                    nc.tensor.matmul(pq[:], xob[:, bass.ts(c, TQ)], wqt[:],
                                     start=True, stop=True)
                    nc.vector.tensor_copy(out=gqT[:, bass.ts(c, DM)], in_=pq[:])
                    pp = psb.tile([TQ, DM], f32, tag="psgq")
                    nc.tensor.matmul(pp[:], poso[:, bass.ts(c, TQ)], wd1t[:],
                                     start=True, stop=True)
                    nc.vector.tensor_copy(out=gpT[:, bass.ts(c, DM)], in_=pp[:])

            # ---------------- Phase B: per query tile ----------------
            with (
                tc.tile_pool(name="score", bufs=3) as spool,
                tc.tile_pool(name="gath", bufs=2) as gpool,
                tc.tile_pool(name="pair", bufs=2) as prpool,
                tc.tile_pool(name="wwuu", bufs=2) as wpool,
                tc.tile_pool(name="topk", bufs=3) as kpool,
                tc.tile_pool(name="tree", bufs=2) as tpool,
                tc.tile_pool(name="ps_s", bufs=3, space="PSUM") as pss,
                tc.tile_pool(name="ps_pair", bufs=1, space="PSUM") as psp,
                tc.tile_pool(name="ps_t", bufs=1, space="PSUM") as pst,
            ):
                for t in range(NT):
                    # ---- scores (f32r: 1 cyc/row on PE) ----
                    sc = spool.tile([TQ, N], f32, tag="sc")
                    for s in range(8):
                        ps = pss.tile([TQ, 512], f32, tag="pssc")
                        nc.tensor.matmul(ps[:],
                                         lhsT65[:, bass.ts(t, TQ)],
                                         rhs65[:, bass.ts(s, 512)],
                                         start=True, stop=True)
                        nc.scalar.activation(sc[:, bass.ts(s, 512)], ps[:], AF.Copy)

                    # ---- top-16 with packed indices ----
                    # per 512-chunk: top-8 values + their in-chunk indices
                    cand = kpool.tile([TQ, NCH * 8], f32, tag="cand")
                    for c in range(NCH):
                        nc.vector.max(out=cand[:, bass.ts(c, 8)],
                                      in_=sc[:, bass.ts(c, CH)])
                    lidx = kpool.tile([TQ, NCH * 8], u16, tag="lidx")
                    for c in range(NCH):
                        nc.vector.max_index(out=lidx[:, bass.ts(c, 8)],
                                            in_max=cand[:, bass.ts(c, 8)],
                                            in_values=sc[:, bass.ts(c, CH)])
                    # global index = chunk*512 + local; pack into low 12
                    # mantissa bits of the candidate value
                    lidxf = kpool.tile([TQ, NCH * 8], f32, tag="lidxf")
                    nc.vector.tensor_copy(out=lidxf[:], in_=lidx[:])
                    gidxf = kpool.tile([TQ, NCH * 8], f32, tag="gidxf")
                    nc.vector.tensor_add(gidxf[:], lidxf[:], choff[:])
                    gidxu = kpool.tile([TQ, NCH * 8], u32, tag="gidxu")
                    nc.vector.tensor_copy(out=gidxu[:], in_=gidxf[:])
                    packed = kpool.tile([TQ, NCH * 8], f32, tag="packed")
                    nc.vector.tensor_tensor(out=packed[:].bitcast(u32),
                                            in0=cand[:].bitcast(u32),
                                            in1=pkmask[:], op=ALU.bitwise_and)
                    nc.vector.tensor_tensor(out=packed[:].bitcast(u32),
                                            in0=packed[:].bitcast(u32),
                                            in1=gidxu[:], op=ALU.bitwise_or)
                    # merge: top-8, kill them, next top-8
                    v8a = kpool.tile([TQ, 8], f32, tag="v8a")
                    nc.vector.max(out=v8a[:], in_=packed[:])
                    repl = kpool.tile([TQ, NCH * 8], f32, tag="repl")
                    nc.vector.match_replace(out=repl[:], in_to_replace=v8a[:],
                                            in_values=packed[:], imm_value=-1e30)
                    v8b = kpool.tile([TQ, 8], f32, tag="v8b")
                    nc.vector.max(out=v8b[:], in_=repl[:])
                    # extract indices from the low bits, replicate into all 8
                    # core groups, transpose so idx16[16c+j, q] = neighbor j.
                    pk16u = kpool.tile([TQ, 16], u32, tag="pk16u")
                    nc.vector.tensor_scalar(out=pk16u[:, 0:8],
                                            in0=v8a[:].bitcast(u32),
                                            scalar1=0xFFF, scalar2=None,
                                            op0=ALU.bitwise_and)
                    nc.vector.tensor_scalar(out=pk16u[:, 8:16],
                                            in0=v8b[:].bitcast(u32),
                                            scalar1=0xFFF, scalar2=None,
                                            op0=ALU.bitwise_and)
                    idxf = kpool.tile([TQ, DM], f32, tag="idxf")
                    nc.vector.tensor_copy(out=idxf[:, 0:16], in_=pk16u[:])
                    nc.vector.tensor_copy(out=idxf[:, 16:32], in_=idxf[:, 0:16])
                    nc.vector.tensor_copy(out=idxf[:, 32:64], in_=idxf[:, 0:32])
                    nc.vector.tensor_copy(out=idxf[:, 64:128], in_=idxf[:, 0:64])
                    pt = pst.tile([DM, TQ], f32, tag="pst")
                    nc.tensor.transpose(pt[:], idxf[:], ident[:])
                    idx16 = kpool.tile([TQ, TQ], i16, tag="idx16")
                    nc.vector.tensor_copy(out=idx16[:], in_=pt[:])

                    # ---- gather [gk | v | gp] from the SBUF table ----
                    gkv = []
                    for gh in range(4):
                        gt = gpool.tile([DM, 3, 512], f16, tag=f"gkv{gh}")
                        nc.gpsimd.dma_gather(
                            out_ap=gt[:], in_ap=table[:],
                            idxs_ap=idx16[:, bass.ts(gh, 32)],
                            num_idxs=512, num_idxs_reg=512, elem_size=ROW,
                            transpose=True,
                            sbuf_tokens_per_rank=128,
                            sbuf_free_dim_per_rank=ROW * 2)
                        gkv.append(gt)

                    # ---- pair MLPs + softmax, in two 1024-pair halves ----
                    # (2-bank PSUM tiles double-buffer so PE works on one half
                    # while ACT/DVE process the other)
                    HP = PAIR // 2
                    for half in range(2):
                        g0, g1 = 2 * half, 2 * half + 1

                        zp = psp.tile([DM, HP], f32, tag="zpair")
                        for hh in range(2):
                            nc.tensor.matmul(zp[:, bass.ts(hh, 512)],
                                             gpT[64 * half:64 * half + 64, bass.ts(t, DM)],
                                             emat[64 * half:64 * half + 64, bass.ts(hh, 512)],
                                             start=True, stop=False)
                        for hh, gh in ((0, g0), (1, g1)):
                            nc.tensor.matmul(zp[:, bass.ts(hh, 512)], negi[:],
                                             gkv[gh][:, 2, :], start=False, stop=True)
                        h1 = prpool.tile([DM, HP], bft, tag="h1")
                        nc.scalar.activation(h1[:], zp[:], AF.Prelu, bias=bd1[:],
                                             scale=1.0, alpha=0.2)
                        zp = psp.tile([DM, HP], f32, tag="zpair")
                        for hh in range(2):
                            sl = bass.ts(hh, 512)
                            nc.tensor.matmul(zp[:, sl], wd2t[:], h1[:, sl],
                                             start=True, stop=True)
                        pe = prpool.tile([DM, HP], bft, tag="pe")
                        nc.scalar.activation(pe[:], zp[:], AF.Prelu, bias=bd2[:],
                                             scale=1.0, alpha=0.2)

                        zp = psp.tile([DM, HP], f32, tag="zpair")
                        for hh in range(2):
                            nc.tensor.matmul(zp[:, bass.ts(hh, 512)],
                                             gqT[64 * half:64 * half + 64, bass.ts(t, DM)],
                                             emat[64 * half:64 * half + 64, bass.ts(hh, 512)],
                                             start=True, stop=False)
                        for hh, gh in ((0, g0), (1, g1)):
                            nc.tensor.matmul(zp[:, bass.ts(hh, 512)], negi[:],
                                             gkv[gh][:, 0, :], start=False, stop=False)
                        for hh in range(2):
                            sl = bass.ts(hh, 512)
                            nc.tensor.matmul(zp[:, sl], wg1t[:], pe[:, sl],
                                             start=False, stop=True)
                        a1 = prpool.tile([DM, HP], bft, tag="a1")
                        nc.scalar.activation(a1[:], zp[:], AF.Prelu, bias=bg1[:],
                                             scale=1.0, alpha=0.2)
                        zp = psp.tile([DM, HP], f32, tag="zpair")
                        for hh in range(2):
                            sl = bass.ts(hh, 512)
                            nc.tensor.matmul(zp[:, sl], wg2t[:], a1[:, sl],
                                             start=True, stop=True)
                        a2 = prpool.tile([DM, HP], bft, tag="a2")
                        nc.scalar.activation(a2[:], zp[:], AF.Prelu, bias=bg2[:],
                                             scale=1.0, alpha=0.2)
                        ee = prpool.tile([DM, HP], bft, tag="ee")
                        nc.scalar.activation(ee[:], a2[:], AF.Exp, bias=0.0,
                                             scale=1.0 / 64.0)

                        # softmax-normalized weighted sum for this half
                        def tree_sum(src_t, nq):
                            e3 = src_t[:].rearrange("p (q k) -> p q k", k=16)
                            t1 = tpool.tile([DM, nq * 8], f32, tag="tr1")
                            nc.vector.tensor_tensor(
                                out=t1[:].rearrange("p (q k) -> p q k", k=8),
                                in0=e3[:, :, 0:8], in1=e3[:, :, 8:16], op=ALU.add)
                            t13 = t1[:].rearrange("p (q k) -> p q k", k=8)
                            t2 = tpool.tile([DM, nq * 4], f32, tag="tr2")
                            nc.vector.tensor_tensor(
                                out=t2[:].rearrange("p (q k) -> p q k", k=4),
                                in0=t13[:, :, 0:4], in1=t13[:, :, 4:8], op=ALU.add)
                            t23 = t2[:].rearrange("p (q k) -> p q k", k=4)
                            t3 = tpool.tile([DM, nq * 2], f32, tag="tr3")
                            nc.vector.tensor_tensor(
                                out=t3[:].rearrange("p (q k) -> p q k", k=2),
                                in0=t23[:, :, 0:2], in1=t23[:, :, 2:4], op=ALU.add)
                            t33 = t3[:].rearrange("p (q k) -> p q k", k=2)
                            t4 = tpool.tile([DM, nq], f32, tag="tr4")
                            nc.vector.tensor_tensor(
                                out=t4[:].rearrange("p (q k) -> p q k", k=1),
                                in0=t33[:, :, 0:1], in1=t33[:, :, 1:2], op=ALU.add)
                            return t4

                        ssum = tree_sum(ee, 64)
                        rrec = tpool.tile([DM, 64], f32, tag="rrec")
                        nc.vector.reciprocal(rrec[:], ssum[:])
                        ww = wpool.tile([DM, HP], bft, tag="ww")
                        for hh, gh in ((0, g0), (1, g1)):
                            nc.vector.tensor_add(ww[:, bass.ts(hh, 512)],
                                                 gkv[gh][:, 1, :],
                                                 pe[:, bass.ts(hh, 512)])
                        uu = wpool.tile([DM, HP], bft, tag="uu")
                        if UU_ON_POOL:
                            nc.gpsimd.tensor_tensor(out=uu[:], in0=ee[:], in1=ww[:],
                                                    op=ALU.mult)
                        else:
                            nc.vector.tensor_mul(uu[:], ee[:], ww[:])
                        ru = tree_sum(uu, 64)
                        nc.vector.tensor_mul(res_all[:, bass.ts(2 * t + half, 64)],
                                             ru[:], rrec[:])

            # ---------------- Phase C: output ----------------
            with (
                tc.tile_pool(name="outp", bufs=2) as opool,
                tc.tile_pool(name="ps_o", bufs=2, space="PSUM") as pso,
            ):
                o1 = opool.tile([DP, NQ], f32, tag="o1")
                for s in range(4):
                    ps = pso.tile([DP, 512], f32, tag="pso")
                    nc.tensor.matmul(ps[:], w2t[:], res_all[:, bass.ts(s, 512)],
                                     start=True, stop=True)
                    nc.scalar.activation(o1[:, bass.ts(s, 512)], ps[:], AF.Prelu,
                                         bias=b2[:], scale=1.0, alpha=0.2)
                o2 = opool.tile([DP, NQ], f32, tag="o2")
                nc.vector.tensor_add(o2[:], o1[:], lhsT65[0:DP, :].bitcast(f32))
                nc.sync.dma_start(out=out_d, in_=o2[:])

    nc.compile()
    return nc


def _host_prep(inputs):
    """Fold BN into weights, build per-core input maps."""
    s1, b1 = _fold_bn(np.asarray(inputs["bn1"]))
    sd1, bd1 = _fold_bn(np.asarray(inputs["bnd1"]))
    sd2, bd2 = _fold_bn(np.asarray(inputs["bnd2"]))
    sg1, bg1 = _fold_bn(np.asarray(inputs["bng1"]))
    sg2, bg2 = _fold_bn(np.asarray(inputs["bng2"]))
    s2, b2 = _fold_bn(np.asarray(inputs["bn2"]))
    W1f = np.asarray(inputs["W1"]) * s1[:, None]
    Wd1f = np.asarray(inputs["Wd1"]) * sd1[:, None]
    Wd2f = np.asarray(inputs["Wd2"]) * sd2[:, None]
    Wg1f = np.asarray(inputs["Wg1"]) * sg1[:, None]
    Wg2f = np.asarray(inputs["Wg2"]) * sg2[:, None]
    W2f = np.asarray(inputs["W2"]) * s2[:, None]
    Wg1k = (Wg1f @ np.asarray(inputs["Wk"])).astype(np.float32)
    Wg1q = (Wg1f @ np.asarray(inputs["Wq"])).astype(np.float32)
    Wv = np.asarray(inputs["Wv"], np.float32)

    E = np.zeros((64, PAIR // 2), np.float32)
    for q in range(64):
        E[q, q * K:(q + 1) * K] = 1.0
    E = np.tile(E, (2, 1))  # same pattern at partitions 0-63 and 64-127

    choff = np.zeros((TQ, NCH * 8), np.float32)
    for c in range(NCH):
        choff[:, c * 8:(c + 1) * 8] = c * CH

    com = {
        "W1fT": np.ascontiguousarray(W1f.T, dtype=bf16),
        "WgkvT": np.ascontiguousarray(
            np.concatenate([Wg1k.T, Wv.T], axis=1), dtype=bf16),
        "Wg1qT": np.ascontiguousarray(Wg1q.T, dtype=bf16),
        "Wd1fT": np.ascontiguousarray(Wd1f.T, dtype=bf16),
        "Wd2fT": np.ascontiguousarray(Wd2f.T, dtype=bf16),
        "Wg1fT": np.ascontiguousarray(Wg1f.T, dtype=bf16),
        "Wg2fT": np.ascontiguousarray(Wg2f.T, dtype=bf16),
        "W2fT": np.ascontiguousarray(W2f.T, dtype=bf16),
        "E": E.astype(bf16),
        "negI": (-np.eye(DM)).astype(np.float16),
        "ident": np.eye(DM, dtype=np.float32),
        "choff": choff,
        "pkmask": np.full((TQ, NCH * 8), 0xFFFFF000, np.uint32),
        "b1": b1.reshape(DM, 1),
        "bd1": bd1.reshape(DM, 1),
        "bd2": bd2.reshape(DM, 1),
        "bg1": bg1.reshape(DM, 1),
        "bg2": bg2.reshape(DM, 1),
        "b2": b2.reshape(DP, 1),
    }

    feats = np.asarray(inputs["feats"], np.float32)
    pos = np.asarray(inputs["pos"], np.float32)
    in_maps = []
    for c in range(8):
        b, h = c // 2, c % 2
        n0 = h * NQ
        fb = feats[b]
        l65 = np.empty((DP + 1, NQ), np.float32)
        l65[0:DP] = fb[:, n0:n0 + NQ]
        l65[DP] = 1.0
        m = dict(com)
        m["feats_f32"] = np.ascontiguousarray(fb)
        m["feats_bf"] = np.ascontiguousarray(fb, dtype=bf16)
        m["fb_own"] = np.ascontiguousarray(fb[:, n0:n0 + NQ], dtype=bf16)
        m["lhsT65"] = l65
        m["pos_bf"] = np.ascontiguousarray(pos[b], dtype=bf16)
        m["pos_own"] = np.ascontiguousarray(pos[b][:, n0:n0 + NQ], dtype=bf16)
        in_maps.append(m)
    return in_maps


def kernel(**inputs):
    from concourse.bass_utils import run_bass_kernel_spmd

    if "nc" not in _CACHE:
        _CACHE["nc"] = _build_bass()
    nc = _CACHE["nc"]
    in_maps = _host_prep(inputs)
    r = run_bass_kernel_spmd(nc, in_maps, core_ids=list(range(8)),
                             **_CACHE.get("run_kwargs", {}))
    _CACHE["last_result"] = r
    out = np.empty((B, DP, N), np.float32)
    for c in range(8):
        b, h = c // 2, c % 2
        out[b][:, h * NQ:(h + 1) * NQ] = r.results[c]["out"]
    return out


# revision 10
# speedup vs baseline: 1.2358x; 1.1264x over previous
"""DGT block (dynamic graph transformer) Bass kernel for Trainium2.

Sharding: 8 cores = 4 batches x 2 query-halves. Each core handles one
batch's feats/pos and one half (2048) of the queries:
  - kNN scores s(n,j) = f_n . f_j - 0.5||f_j||^2  (rank-equivalent to -dist/2)
    computed as float32r on PE (1 cyc/row vs 4 for fp32); top-16 via
    per-512-chunk max8 + max_index8, with the global column index packed
    into the low 12 mantissa bits of each candidate so the merge
    (max8 / match_replace / max8) carries indices along for free.
  - gather table rows [gk | v | gp] in f16 kept in SBUF (24KB/partition);
    fetched per query tile with gpsimd dma_gather(transpose=True,
    SBUF source) so channels land on partitions.
  - attention MLPs as bf16 matmuls with PSUM accumulation; the per-query
    broadcast terms (gq_n, gp_n) enter via an expander matmul and the
    gathered terms (gk_j, gp_j) via a (-I) matmul.
  - per-channel softmax over the 16 neighbors: pairwise-tree adds on DVE,
    weighted-sum multiply on GpSimd to offload the critical DVE engine.
"""

import numpy as np
import ml_dtypes

B, N, K, DP, DM, EPS = 4, 4096, 16, 64, 128, 1e-5
NQ = N // 2            # queries per core
TQ = 128               # queries per tile
NT = NQ // TQ          # tiles per core (16)
PAIR = TQ * K          # pairs per tile (2048)
CH = 512               # top-k scan chunk size
NCH = N // CH          # 8 chunks
ROW = 3 * DM           # gather-table row elems (f16): [gk(128) | v(128) | gp(128)]

UU_ON_POOL = False     # run the ee*(v+pe) multiply on GpSimd instead of DVE

_CACHE = {}

bf16 = ml_dtypes.bfloat16


def _fold_bn(p):
    g, be, m, v = p.astype(np.float64)
    s = g / np.sqrt(v + EPS)
    return (s).astype(np.float32), (be - m * s).astype(np.float32)


def _build_bass():
    import concourse.bass as bass
    import concourse.mybir as mybir
    import concourse.bacc as bacc
    from concourse.tile import TileContext

    dt = mybir.dt
    AF = mybir.ActivationFunctionType
    ALU = mybir.AluOpType
    AX = mybir.AxisListType

    nc = bacc.Bacc("TRN2", target_bir_lowering=False, debug=False, num_devices=8)

    # ---- I/O ----
    def inp(name, shape, dtype):
        return nc.dram_tensor(name, list(shape), dtype, kind="ExternalInput").ap()

    feats_f32 = inp("feats_f32", (DP, N), dt.float32r)
    feats_bf = inp("feats_bf", (DP, N), dt.bfloat16)
    fb_own = inp("fb_own", (DP, NQ), dt.bfloat16)
    lhsT65_d = inp("lhsT65", (DP + 1, NQ), dt.float32r)
    pos_bf = inp("pos_bf", (3, N), dt.bfloat16)
    pos_own = inp("pos_own", (3, NQ), dt.bfloat16)
    w1t_d = inp("W1fT", (DP, DM), dt.bfloat16)
    wkvt_d = inp("WgkvT", (DM, 2 * DM), dt.bfloat16)
    wqt_d = inp("Wg1qT", (DM, DM), dt.bfloat16)
    wd1t_d = inp("Wd1fT", (3, DM), dt.bfloat16)
    wd2t_d = inp("Wd2fT", (DM, DM), dt.bfloat16)
    wg1t_d = inp("Wg1fT", (DM, DM), dt.bfloat16)
    wg2t_d = inp("Wg2fT", (DM, DM), dt.bfloat16)
    w2t_d = inp("W2fT", (DM, DP), dt.bfloat16)
    e_d = inp("E", (TQ, PAIR // 2), dt.bfloat16)
    negi_d = inp("negI", (DM, DM), dt.float16)
    choff_d = inp("choff", (TQ, NCH * 8), dt.float32)
    pkmask_d = inp("pkmask", (TQ, NCH * 8), dt.uint32)
    b1_d = inp("b1", (DM, 1), dt.float32)
    bd1_d = inp("bd1", (DM, 1), dt.float32)
    bd2_d = inp("bd2", (DM, 1), dt.float32)
    bg1_d = inp("bg1", (DM, 1), dt.float32)
    bg2_d = inp("bg2", (DM, 1), dt.float32)
    b2_d = inp("b2", (DP, 1), dt.float32)

    out_d = nc.dram_tensor("out", [DP, NQ], dt.float32, kind="ExternalOutput").ap()

    f32, f32r = dt.float32, dt.float32r
    f16, bft, i16, u16, u32 = dt.float16, dt.bfloat16, dt.int16, dt.uint16, dt.uint32

    with TileContext(nc) as tc:
        with (
            tc.tile_pool(name="const", bufs=1) as cpool,
            tc.tile_pool(name="persist", bufs=1) as ppool,
        ):
            # persistent constants
            w1t = cpool.tile_from(w1t_d)
            wkvt = cpool.tile_from(wkvt_d)
            wqt = cpool.tile_from(wqt_d)
            wd1t = cpool.tile_from(wd1t_d)
            wd2t = cpool.tile_from(wd2t_d)
            wg1t = cpool.tile_from(wg1t_d)
            wg2t = cpool.tile_from(wg2t_d)
            w2t = cpool.tile_from(w2t_d)
            emat = cpool.tile_from(e_d)
            negi = cpool.tile_from(negi_d)
            choff = cpool.tile_from(choff_d)
            pkmask = cpool.tile_from(pkmask_d)
            b1 = cpool.tile_from(b1_d)
            bd1 = cpool.tile_from(bd1_d)
            bd2 = cpool.tile_from(bd2_d)
            bg1 = cpool.tile_from(bg1_d)
            bg2 = cpool.tile_from(bg2_d)
            b2 = cpool.tile_from(b2_d)

            # persistent working tensors
            rhs65 = ppool.tile([DP + 1, N], f32r)
            lhsT65 = ppool.tile_from(lhsT65_d)
            gqT = ppool.tile([TQ, NQ], bft)   # (q, m) blocks per tile
            gpT = ppool.tile([TQ, NQ], bft)
            res_all = ppool.tile([DM, NQ], bft)
            # gather table in SBUF: token n -> partition n%128, rank n//128
            table = ppool.tile([128, N // 128, ROW], f16)

            # ---------------- Phase A: setup ----------------
            with (
                tc.tile_pool(name="setupA", bufs=2) as apool,
                tc.tile_pool(name="xpool", bufs=1) as xpool,
                tc.tile_pool(name="ps_a", bufs=2, space="PSUM") as psa,
                tc.tile_pool(name="ps_b", bufs=1, space="PSUM") as psb,
            ):
                nc.sync.dma_start(out=rhs65[0:DP, :], in_=feats_f32)

                fbt = xpool.tile([DP, N], bft)
                nc.sync.dma_start(out=fbt[:], in_=feats_bf)
                post = xpool.tile([3, N], bft)
                nc.sync.dma_start(out=post[:], in_=pos_bf)
                xob = xpool.tile([DM, NQ], bft)
                fob = xpool.tile([DP, NQ], bft)
                nc.sync.dma_start(out=fob[:], in_=fb_own)
                poso = xpool.tile([3, NQ], bft)
                nc.sync.dma_start(out=poso[:], in_=pos_own)

                ones64 = cpool.tile([DP, 1], f32)
                nc.vector.memset(ones64[:], 1.0)

                # row 64 of rhs65 first: the tile-0 score matmuls (and the
                # whole DVE top-k pipeline) only gate on rhs65/lhsT65, so
                # finishing this before x/table production lets Phase B start
                # ~25us earlier.
                ff = apool.tile([DP, 512], f32, tag="ff")
                for s in range(8):
                    nc.vector.tensor_mul(ff[:], rhs65[0:DP, bass.ts(s, 512)].bitcast(f32),
                                         rhs65[0:DP, bass.ts(s, 512)].bitcast(f32))
                    ps = psb.tile([1, 512], f32, tag="pssq")
                    nc.tensor.matmul(ps[:], ones64[:], ff[:], start=True, stop=True)
                    nc.scalar.activation(rhs65[DP:DP + 1, bass.ts(s, 512)], ps[:],
                                         AF.Copy, bias=0.0, scale=-0.5)
                    ff = apool.tile([DP, 512], f32, tag="ff")

                xfull = xpool.tile([DM, N], bft)
                for s in range(8):
                    ps = psa.tile([DM, 512], f32, tag="psx")
                    nc.tensor.matmul(ps[:], w1t[:], fbt[:, bass.ts(s, 512)],
                                     start=True, stop=True)
                    nc.scalar.activation(xfull[:, bass.ts(s, 512)], ps[:],
                                         AF.Prelu, bias=b1[:], scale=1.0, alpha=0.2)
                # x for own queries (recomputed from the own slice input)
                for s in range(4):
                    ps = psa.tile([DM, 512], f32, tag="psx")
                    nc.tensor.matmul(ps[:], w1t[:], fob[:, bass.ts(s, 512)],
                                     start=True, stop=True)
                    nc.scalar.activation(xob[:, bass.ts(s, 512)], ps[:],
                                         AF.Prelu, bias=b1[:], scale=1.0, alpha=0.2)

                # gather table rows [gk | v | gp] in f16, written straight
                # into the SBUF-resident table (token n at partition n%128,
                # rank n//128).
                for c in range(32):
                    pkv = psa.tile([TQ, 2 * DM], f32, tag="pskv")
                    nc.tensor.matmul(pkv[:], xfull[:, bass.ts(c, TQ)], wkvt[:],
                                     start=True, stop=True)
                    pgp = psb.tile([TQ, DM], f32, tag="psgp")
                    nc.tensor.matmul(pgp[:], post[:, bass.ts(c, TQ)], wd1t[:],
                                     start=True, stop=True)
                    eng = nc.vector if (c % 2 == 0) else nc.scalar
                    if eng is nc.vector:
                        nc.vector.tensor_copy(out=table[:, c, 0:2 * DM], in_=pkv[:])
                        nc.vector.tensor_copy(out=table[:, c, 2 * DM:ROW], in_=pgp[:])
                    else:
                        nc.scalar.activation(table[:, c, 0:2 * DM], pkv[:], AF.Copy)
                        nc.scalar.activation(table[:, c, 2 * DM:ROW], pgp[:], AF.Copy)

                # gqT / gpT for own queries
                for c in range(NT):
                    pq = psb.tile([TQ, DM], f32, tag="psgq")
                    nc.tensor.matmul(pq[:], xob[:, bass.ts(c, TQ)], wqt[:],
                                     start=True, stop=True)
                    nc.vector.tensor_copy(out=gqT[:, bass.ts(c, DM)], in_=pq[:])
                    pp = psb.tile([TQ, DM], f32, tag="psgq")
                    nc.tensor.matmul(pp[:], poso[:, bass.ts(c, TQ)], wd1t[:],
                                     start=True, stop=True)
                    nc.vector.tensor_copy(out=gpT[:, bass.ts(c, DM)], in_=pp[:])

            # ---------------- Phase B: per query tile ----------------
            with (
                tc.tile_pool(name="score", bufs=3) as spool,
                tc.tile_pool(name="gath", bufs=2) as gpool,
                tc.tile_pool(name="pair", bufs=2) as prpool,
                tc.tile_pool(name="wwuu", bufs=2) as wpool,
                tc.tile_pool(name="topk", bufs=3) as kpool,
                tc.tile_pool(name="tree", bufs=2) as tpool,
                tc.tile_pool(name="ps_s", bufs=1, space="PSUM") as pss,
                tc.tile_pool(name="ps_pair", bufs=2, space="PSUM") as psp,
            ):
                for t in range(NT):
                    # ---- scores (f32r: 1 cyc/row on PE) ----
                    sc = spool.tile([TQ, N], f32, tag="sc")
                    for hs in range(2):
                        ps = pss.tile([TQ, 2048], f32, tag="pssc")
                        for s in range(4):
                            nc.tensor.matmul(ps[:, bass.ts(s, 512)],
                                             lhsT65[:, bass.ts(t, TQ)],
                                             rhs65[:, bass.ts(4 * hs + s, 512)],
                                             start=True, stop=True)
                        nc.scalar.activation(sc[:, bass.ts(hs, 2048)], ps[:], AF.Copy)

                    # ---- top-16 with packed indices ----
                    # per 512-chunk: top-8 values + their in-chunk indices
                    cand = kpool.tile([TQ, NCH * 8], f32, tag="cand")
                    for c in range(NCH):
                        nc.vector.max(out=cand[:, bass.ts(c, 8)],
                                      in_=sc[:, bass.ts(c, CH)])
                    lidx = kpool.tile([TQ, NCH * 8], u16, tag="lidx")
                    for c in range(NCH):
                        nc.vector.max_index(out=lidx[:, bass.ts(c, 8)],
                                            in_max=cand[:, bass.ts(c, 8)],
                                            in_values=sc[:, bass.ts(c, CH)])
                    # global index = chunk*512 + local; pack into low 12
                    # mantissa bits of the candidate value
                    lidxf = kpool.tile([TQ, NCH * 8], f32, tag="lidxf")
                    nc.vector.tensor_copy(out=lidxf[:], in_=lidx[:])
                    gidxf = kpool.tile([TQ, NCH * 8], f32, tag="gidxf")
                    nc.vector.tensor_add(gidxf[:], lidxf[:], choff[:])
                    gidxu = kpool.tile([TQ, NCH * 8], u32, tag="gidxu")
                    nc.vector.tensor_copy(out=gidxu[:], in_=gidxf[:])
                    packed = kpool.tile([TQ, NCH * 8], f32, tag="packed")
                    nc.vector.tensor_tensor(out=packed[:].bitcast(u32),
                                            in0=cand[:].bitcast(u32),
                                            in1=pkmask[:], op=ALU.bitwise_and)
                    nc.vector.tensor_tensor(out=packed[:].bitcast(u32),
                                            in0=packed[:].bitcast(u32),
                                            in1=gidxu[:], op=ALU.bitwise_or)
                    # merge: top-8, kill them, next top-8
                    v8a = kpool.tile([TQ, 8], f32, tag="v8a")
                    nc.vector.max(out=v8a[:], in_=packed[:])
                    repl = kpool.tile([TQ, NCH * 8], f32, tag="repl")
                    nc.vector.match_replace(out=repl[:], in_to_replace=v8a[:],
                                            in_values=packed[:], imm_value=-1e30)
                    v8b = kpool.tile([TQ, 8], f32, tag="v8b")
                    nc.vector.max(out=v8b[:], in_=repl[:])
                    # extract indices from the low bits, replicate into all 8
                    # core groups, transpose so idx16[16c+j, q] = neighbor j.
                    pk16u = kpool.tile([TQ, 16], u32, tag="pk16u")
                    nc.vector.tensor_scalar(out=pk16u[:, 0:8],
                                            in0=v8a[:].bitcast(u32),
                                            scalar1=0xFFF, scalar2=None,
                                            op0=ALU.bitwise_and)
                    nc.vector.tensor_scalar(out=pk16u[:, 8:16],
                                            in0=v8b[:].bitcast(u32),
                                            scalar1=0xFFF, scalar2=None,
                                            op0=ALU.bitwise_and)
                    idxr = kpool.tile([TQ, DM], i16, tag="idxr")
                    nc.vector.tensor_copy(out=idxr[:, 0:16], in_=pk16u[:])
                    nc.vector.tensor_copy(out=idxr[:, 16:32], in_=idxr[:, 0:16])
                    nc.vector.tensor_copy(out=idxr[:, 32:64], in_=idxr[:, 0:32])
                    nc.vector.tensor_copy(out=idxr[:, 64:128], in_=idxr[:, 0:64])
                    idx16 = kpool.tile([TQ, TQ], i16, tag="idx16")
                    nc.sync.dma_start_transpose(idx16[:], idxr[:])

                    # ---- gather [gk | v | gp] from the SBUF table ----
                    gkv = []
                    for gh in range(4):
                        gt = gpool.tile([DM, 3, 512], f16, tag=f"gkv{gh}")
                        nc.gpsimd.dma_gather(
                            out_ap=gt[:], in_ap=table[:],
                            idxs_ap=idx16[:, bass.ts(gh, 32)],
                            num_idxs=512, num_idxs_reg=512, elem_size=ROW,
                            transpose=True,
                            sbuf_tokens_per_rank=128,
                            sbuf_free_dim_per_rank=ROW * 2)
                        gkv.append(gt)

                    # ---- pair MLPs in two 1024-pair halves (PSUM ping-pong);
                    # softmax tail on full-tile tensors ----
                    HP = PAIR // 2
                    pe = prpool.tile([DM, PAIR], bft, tag="pe")
                    ee = prpool.tile([DM, PAIR], bft, tag="ee")
                    for half in range(2):
                        g0, g1 = 2 * half, 2 * half + 1
                        hsl = bass.ts(half, HP)

                        zp = psp.tile([DM, HP], f32, tag="zpair")
                        for hh in range(2):
                            nc.tensor.matmul(zp[:, bass.ts(hh, 512)],
                                             gpT[64 * half:64 * half + 64, bass.ts(t, DM)],
                                             emat[64 * half:64 * half + 64, bass.ts(hh, 512)],
                                             start=True, stop=False)
                        for hh, gh in ((0, g0), (1, g1)):
                            nc.tensor.matmul(zp[:, bass.ts(hh, 512)], negi[:],
                                             gkv[gh][:, 2, :], start=False, stop=True)
                        h1 = prpool.tile([DM, HP], bft, tag="h1")
                        nc.scalar.activation(h1[:], zp[:], AF.Prelu, bias=bd1[:],
                                             scale=1.0, alpha=0.2)
                        zp = psp.tile([DM, HP], f32, tag="zpair")
                        for hh in range(2):
                            sl = bass.ts(hh, 512)
                            nc.tensor.matmul(zp[:, sl], wd2t[:], h1[:, sl],
                                             start=True, stop=True)
                        nc.scalar.activation(pe[:, hsl], zp[:], AF.Prelu, bias=bd2[:],
                                             scale=1.0, alpha=0.2)

                        zp = psp.tile([DM, HP], f32, tag="zpair")
                        for hh in range(2):
                            nc.tensor.matmul(zp[:, bass.ts(hh, 512)],
                                             gqT[64 * half:64 * half + 64, bass.ts(t, DM)],
                                             emat[64 * half:64 * half + 64, bass.ts(hh, 512)],
                                             start=True, stop=False)
                        for hh, gh in ((0, g0), (1, g1)):
                            nc.tensor.matmul(zp[:, bass.ts(hh, 512)], negi[:],
                                             gkv[gh][:, 0, :], start=False, stop=False)
                        for hh in range(2):
                            sl = bass.ts(hh, 512)
                            nc.tensor.matmul(zp[:, sl], wg1t[:],
                                             pe[:, hsl][:, sl], start=False, stop=True)
                        a1 = prpool.tile([DM, HP], bft, tag="a1")
                        nc.scalar.activation(a1[:], zp[:], AF.Prelu, bias=bg1[:],
                                             scale=1.0, alpha=0.2)
                        zp = psp.tile([DM, HP], f32, tag="zpair")
                        for hh in range(2):
                            sl = bass.ts(hh, 512)
                            nc.tensor.matmul(zp[:, sl], wg2t[:], a1[:, sl],
                                             start=True, stop=True)
                        a2 = prpool.tile([DM, HP], bft, tag="a2")
                        nc.scalar.activation(a2[:], zp[:], AF.Prelu, bias=bg2[:],
                                             scale=1.0, alpha=0.2)
                        nc.scalar.activation(ee[:, hsl], a2[:], AF.Exp, bias=0.0,
                                             scale=1.0 / 64.0)

                    # ---- softmax-normalized weighted sum (full tile) ----
                    def tree_sum(src_t):
                        e3 = src_t[:].rearrange("p (q k) -> p q k", k=16)
                        t1 = tpool.tile([DM, TQ * 8], f32, tag="tr1")
                        nc.vector.tensor_tensor(
                            out=t1[:].rearrange("p (q k) -> p q k", k=8),
                            in0=e3[:, :, 0:8], in1=e3[:, :, 8:16], op=ALU.add)
                        t13 = t1[:].rearrange("p (q k) -> p q k", k=8)
                        t2 = tpool.tile([DM, TQ * 4], f32, tag="tr2")
                        nc.vector.tensor_tensor(
                            out=t2[:].rearrange("p (q k) -> p q k", k=4),
                            in0=t13[:, :, 0:4], in1=t13[:, :, 4:8], op=ALU.add)
                        t23 = t2[:].rearrange("p (q k) -> p q k", k=4)
                        t3 = tpool.tile([DM, TQ * 2], f32, tag="tr3")
                        nc.vector.tensor_tensor(
                            out=t3[:].rearrange("p (q k) -> p q k", k=2),
                            in0=t23[:, :, 0:2], in1=t23[:, :, 2:4], op=ALU.add)
                        t33 = t3[:].rearrange("p (q k) -> p q k", k=2)
                        t4 = tpool.tile([DM, TQ], f32, tag="tr4")
                        nc.vector.tensor_tensor(
                            out=t4[:].rearrange("p (q k) -> p q k", k=1),
                            in0=t33[:, :, 0:1], in1=t33[:, :, 1:2], op=ALU.add)
                        return t4

                    ssum = tree_sum(ee)
                    rrec = tpool.tile([DM, TQ], f32, tag="rrec")
                    nc.vector.reciprocal(rrec[:], ssum[:])
                    ww = wpool.tile([DM, PAIR], bft, tag="ww")
                    for gh in range(4):
                        nc.vector.tensor_add(ww[:, bass.ts(gh, 512)],
                                             gkv[gh][:, 1, :],
                                             pe[:, bass.ts(gh, 512)])
                    uu = wpool.tile([DM, PAIR], bft, tag="uu")
                    if UU_ON_POOL:
                        nc.gpsimd.tensor_tensor(out=uu[:], in0=ee[:], in1=ww[:],
                                                op=ALU.mult)
                    else:
                        nc.vector.tensor_mul(uu[:], ee[:], ww[:])
                    ru = tree_sum(uu)
                    nc.vector.tensor_mul(res_all[:, bass.ts(t, TQ)], ru[:], rrec[:])

            # ---------------- Phase C: output ----------------
            with (
                tc.tile_pool(name="outp", bufs=2) as opool,
                tc.tile_pool(name="ps_o", bufs=2, space="PSUM") as pso,
            ):
                o1 = opool.tile([DP, NQ], f32, tag="o1")
                for s in range(4):
                    ps = pso.tile([DP, 512], f32, tag="pso")
                    nc.tensor.matmul(ps[:], w2t[:], res_all[:, bass.ts(s, 512)],
                                     start=True, stop=True)
                    nc.scalar.activation(o1[:, bass.ts(s, 512)], ps[:], AF.Prelu,
                                         bias=b2[:], scale=1.0, alpha=0.2)
                o2 = opool.tile([DP, NQ], f32, tag="o2")
                nc.vector.tensor_add(o2[:], o1[:], lhsT65[0:DP, :].bitcast(f32))
                nc.sync.dma_start(out=out_d, in_=o2[:])

    nc.compile()
    return nc


def _host_prep(inputs):
    """Fold BN into weights, build per-core input maps."""
    s1, b1 = _fold_bn(np.asarray(inputs["bn1"]))
    sd1, bd1 = _fold_bn(np.asarray(inputs["bnd1"]))
    sd2, bd2 = _fold_bn(np.asarray(inputs["bnd2"]))
    sg1, bg1 = _fold_bn(np.asarray(inputs["bng1"]))
    sg2, bg2 = _fold_bn(np.asarray(inputs["bng2"]))
    s2, b2 = _fold_bn(np.asarray(inputs["bn2"]))
    W1f = np.asarray(inputs["W1"]) * s1[:, None]
    Wd1f = np.asarray(inputs["Wd1"]) * sd1[:, None]
    Wd2f = np.asarray(inputs["Wd2"]) * sd2[:, None]
    Wg1f = np.asarray(inputs["Wg1"]) * sg1[:, None]
    Wg2f = np.asarray(inputs["Wg2"]) * sg2[:, None]
    W2f = np.asarray(inputs["W2"]) * s2[:, None]
    Wg1k = (Wg1f @ np.asarray(inputs["Wk"])).astype(np.float32)
    Wg1q = (Wg1f @ np.asarray(inputs["Wq"])).astype(np.float32)
    Wv = np.asarray(inputs["Wv"], np.float32)

    E = np.zeros((64, PAIR // 2), np.float32)
    for q in range(64):
        E[q, q * K:(q + 1) * K] = 1.0
    E = np.tile(E, (2, 1))  # same pattern at partitions 0-63 and 64-127

    choff = np.zeros((TQ, NCH * 8), np.float32)
    for c in range(NCH):
        choff[:, c * 8:(c + 1) * 8] = c * CH

    com = {
        "W1fT": np.ascontiguousarray(W1f.T, dtype=bf16),
        "WgkvT": np.ascontiguousarray(
            np.concatenate([Wg1k.T, Wv.T], axis=1), dtype=bf16),
        "Wg1qT": np.ascontiguousarray(Wg1q.T, dtype=bf16),
        "Wd1fT": np.ascontiguousarray(Wd1f.T, dtype=bf16),
        "Wd2fT": np.ascontiguousarray(Wd2f.T, dtype=bf16),
        "Wg1fT": np.ascontiguousarray(Wg1f.T, dtype=bf16),
        "Wg2fT": np.ascontiguousarray(Wg2f.T, dtype=bf16),
        "W2fT": np.ascontiguousarray(W2f.T, dtype=bf16),
        "E": E.astype(bf16),
        "negI": (-np.eye(DM)).astype(np.float16),
        "choff": choff,
        "pkmask": np.full((TQ, NCH * 8), 0xFFFFF000, np.uint32),
        "b1": b1.reshape(DM, 1),
        "bd1": bd1.reshape(DM, 1),
        "bd2": bd2.reshape(DM, 1),
        "bg1": bg1.reshape(DM, 1),
        "bg2": bg2.reshape(DM, 1),
        "b2": b2.reshape(DP, 1),
    }

    feats = np.asarray(inputs["feats"], np.float32)
    pos = np.asarray(inputs["pos"], np.float32)
    in_maps = []
    for c in range(8):
        b, h = c // 2, c % 2
        n0 = h * NQ
        fb = feats[b]
        l65 = np.empty((DP + 1, NQ), np.float32)
        l65[0:DP] = fb[:, n0:n0 + NQ]
        l65[DP] = 1.0
        m = dict(com)
        m["feats_f32"] = np.ascontiguousarray(fb)
        m["feats_bf"] = np.ascontiguousarray(fb, dtype=bf16)
        m["fb_own"] = np.ascontiguousarray(fb[:, n0:n0 + NQ], dtype=bf16)
        m["lhsT65"] = l65
        m["pos_bf"] = np.ascontiguousarray(pos[b], dtype=bf16)
        m["pos_own"] = np.ascontiguousarray(pos[b][:, n0:n0 + NQ], dtype=bf16)
        in_maps.append(m)
    return in_maps


def kernel(**inputs):
    from concourse.bass_utils import run_bass_kernel_spmd

    if "nc" not in _CACHE:
        _CACHE["nc"] = _build_bass()
    nc = _CACHE["nc"]
    in_maps = _host_prep(inputs)
    r = run_bass_kernel_spmd(nc, in_maps, core_ids=list(range(8)),
                             **_CACHE.get("run_kwargs", {}))
    _CACHE["last_result"] = r
    out = np.empty((B, DP, N), np.float32)
    for c in range(8):
        b, h = c // 2, c % 2
        out[b][:, h * NQ:(h + 1) * NQ] = r.results[c]["out"]
    return out
